# revision 4
# baseline (speedup 1.0000x reference)
"""GroupedQueryAttention on 8 Trainium2 NeuronCores — bf16, pipeline-tuned.

Problem (hardcoded): B=2, T=2048, DIM=4096, 32 q heads, 8 kv heads, hd=128.
  q = x @ Wq.T ; k,v = split(x @ Wkv.T) ; causal softmax(q k^T/sqrt(hd)) v ; out = o @ Wo.T

Sharding: hybrid data x tensor parallel over 8 cores.
  core c -> batch b = c//4, kv-head group j = c%4 (kv heads {2j,2j+1}, q heads {8j..8j+7}).
Per core:
  phase 1: QT/KT/VT projections, tg-major with x held SBUF-resident in rotating
           quarter-slabs ([128,8,512] x6 bufs) so matmuls never wait on or
           contend with a per-tile DMA stream (measured 246 -> 220 ns/matmul).
           x is read from HBM exactly once. V transposes run inside the last tg.
  phase 2: flash-style causal attention per q head in scores-TRANSPOSED layout
           sT[k,q] = KT_tile.T @ QT. Diagonal k-tiles compute only the causal
           column range (q >= k); the dead left region of sT is overwritten
           with -1e30 by a DVE mask copy, and exp runs full-width so masked
           lanes land as exact zeros for the denominator adds.
           Denominators via an all-ones[128,128] matmul of DVE-pre-added quads:
           the matmul broadcasts the row of sums to all 128 partitions, so
           normalization is recip+mult on DVE only and the gpsimd queue carries
           nothing but collective triggers (a rendezvous wait there stalls no
           compute). Heads iterate tg-MAJOR: all 8 heads finish t-group
           tg before tg+1 starts, so the (head, tg) AllGathers for tg<3 have
           long drained when phase 3 consumes them -- the gather tail hides
           behind phase-3 compute on earlier t-groups.
  phase 3: outT slice = WoT.T @ oT_full with the gathered oT staged through the
           same rotating quarter-slab scheme (og read from DRAM once per tg).
Host: output slices are disjoint; just transpose/concat.
"""

import sys

sys.path.insert(0, "/opt/trn_rl_repo")

import math

import numpy as np

import concourse.bass as bass
import concourse.bacc as bacc
import concourse.tile as tile
from concourse import mybir
from concourse.bass_utils import run_bass_kernel_spmd

B, T, DIM = 2, 2048, 4096
N_HEADS, N_KV, HD = 32, 8, 128
R = N_HEADS // N_KV  # 4
NCORES = 8
GROUPS = [[0, 1, 2, 3], [4, 5, 6, 7]]

HPC = 8  # q heads per core
KVPC = 2  # kv heads per core
EQ = HPC * HD  # 1024 q-proj out features per core
EKV = KVPC * HD  # 256 k (and v) out features per core
NT = T // 512  # 4 t-groups of 512
NC = DIM // 128  # 32 contraction tiles
NKB = T // 128  # 16 k-tiles per head

BF = mybir.dt.bfloat16
F32 = mybir.dt.float32
INV_SQRT_HD = 1.0 / math.sqrt(HD)


def build():
    nc = bacc.Bacc("TRN2", num_devices=NCORES)

    # ---- external I/O (per-core data differs, program is SPMD-identical) ----
    xT = nc.dram_tensor("xT", [DIM, T], BF, kind="ExternalInput")  # x[b].T
    wallT = nc.dram_tensor("wallT", [DIM, EQ + 2 * EKV], BF, kind="ExternalInput")
    woT = nc.dram_tensor("woT", [DIM, EQ], BF, kind="ExternalInput")  # Wo[oc_slice,:].T
    # maskL: [:,0:128] causal triangle add-mask, [:,128:512] all -1e30 copy-mask
    maskL = nc.dram_tensor("maskL", [128, 512], F32, kind="ExternalInput")
    ident = nc.dram_tensor("ident", [128, 128], BF, kind="ExternalInput")
    ones_in = nc.dram_tensor("ones_in", [128, 128], BF, kind="ExternalInput")
    out_part = nc.dram_tensor("out_part", [EQ, T], F32, kind="ExternalOutput")

    EALL = EQ + 2 * EKV  # 1536, 12 e-tiles: 8 Q, 2 K, 2 V

    with tile.TileContext(nc) as tc:
        with (
            tc.tile_pool(name="persist", bufs=1) as persist,
            tc.tile_pool(name="dram2", bufs=1, space="DRAM") as dram2,
        ):
            # ---------------- constants ----------------
            mask_sb = persist.tile([128, 512], F32)
            nc.sync.dma_start(out=mask_sb[:], in_=maskL[:, :])
            ident_sb = persist.tile([128, 128], BF)
            nc.sync.dma_start(out=ident_sb[:], in_=ident[:, :])
            ones_sb = persist.tile([128, 128], BF)
            nc.sync.dma_start(out=ones_sb[:], in_=ones_in[:, :])

            # persistent activations
            qt_sb = persist.tile([128, HPC * T], BF)  # QT: head h at cols [h*T,(h+1)*T)
            kt_sb = persist.tile([128, KVPC * T], BF)  # KT per kv head
            vt_sb = persist.tile([128, KVPC * T], BF)  # VT per kv head
            v_sb = persist.tile([128, KVPC * T], BF)  # V[t,dv]: tile (g,kb) at (g*16+kb)*128

            # per-(head, tg) AllGather buffers
            og_in = [[None] * NT for _ in range(HPC)]
            og_out = [[None] * NT for _ in range(HPC)]
            for h in range(HPC):
                for tg in range(NT):
                    og_in[h][tg] = dram2.tile([128, 512], BF, name=f"og_in_{h}_{tg}")
                    og_out[h][tg] = dram2.tile(
                        [4 * 128, 512], BF, name=f"og_out_{h}_{tg}"
                    )

            with (
                tc.tile_pool(name="wall_pool", bufs=1) as wall_pool,
                tc.tile_pool(name="xs_pool", bufs=6) as xs_pool,
                tc.tile_pool(name="psum_p1", bufs=2, space="PSUM") as psum_p1,
            ):
                # phase-1 weights: c-tile cb at cols [cb*EALL, (cb+1)*EALL)
                wall_sb = wall_pool.tile([128, NC * EALL], BF)

                # e-tile order: K0 K1 V0 V1 first so attention deps clear early
                etile_order = [HPC, HPC + 1, HPC + 2, HPC + 3] + list(range(HPC))

                def etile_dst(e):
                    # e indexes [Q0..Q7, K0, K1, V0, V1]
                    if e < HPC:
                        return qt_sb[:, e * T:(e + 1) * T]
                    if e < HPC + KVPC:
                        g = e - HPC
                        return kt_sb[:, g * T:(g + 1) * T]
                    g = e - HPC - KVPC
                    return vt_sb[:, g * T:(g + 1) * T]

                # ---------------- phase 1: projections ----------------
                for tg in range(NT):
                    quarters = []
                    for q in range(4):
                        xq = xs_pool.tile([128, 8, 512], BF, tag="xs",
                                          name=f"xs_{tg}_{q}")
                        for i in range(8):
                            cb = q * 8 + i
                            nc.sync.dma_start(
                                out=xq[:, i, :],
                                in_=xT[cb * 128:(cb + 1) * 128,
                                       tg * 512:(tg + 1) * 512],
                            )
                            if tg == 0:
                                nc.sync.dma_start(
                                    out=wall_sb[:, cb * EALL:(cb + 1) * EALL],
                                    in_=wallT[cb * 128:(cb + 1) * 128, :],
                                )
                        quarters.append(xq)
                    for chunk in range(3):  # 3 chunks of 4 e-tiles
                        es = etile_order[chunk * 4:(chunk + 1) * 4]
                        accs = []
                        for i, e in enumerate(es):
                            acc = psum_p1.tile([128, 512], F32, tag=f"acc{i}",
                                               name=f"p1acc{i}")
                            accs.append(acc)
                        for cb in range(NC):
                            xt_sl = quarters[cb // 8][:, cb % 8, :]
                            for i, e in enumerate(es):
                                nc.tensor.matmul(
                                    accs[i][:],
                                    wall_sb[:, cb * EALL + e * 128:
                                            cb * EALL + (e + 1) * 128],
                                    xt_sl,
                                    start=(cb == 0),
                                    stop=(cb == NC - 1),
                                )
                        for i, e in enumerate(es):
                            nc.vector.tensor_copy(
                                etile_dst(e)[:, tg * 512:(tg + 1) * 512], accs[i][:]
                            )
                        if chunk == 0:
                            # V = VT.T for this tg's k-tiles (PE transpose-mode)
                            for g in range(KVPC):
                                for kb in range(4 * tg, 4 * tg + 4):
                                    tp = psum_p1.tile([128, 128], BF, tag="acc0",
                                                      name="vtp")
                                    nc.tensor.transpose(
                                        tp[:],
                                        vt_sb[:, g * T + kb * 128:
                                              g * T + (kb + 1) * 128],
                                        ident_sb[:],
                                    )
                                    nc.vector.tensor_copy(
                                        v_sb[:, (g * NKB + kb) * 128:
                                             (g * NKB + kb + 1) * 128],
                                        tp[:],
                                    )

            # wall/xs/psum_p1 released; phase 2/3 reuse that SBUF/PSUM space.
            with (
                tc.tile_pool(name="p23", bufs=1) as p23,
                tc.tile_pool(name="work2", bufs=3) as work2,
            ):
                woT_sb = p23.tile([128, NC * EQ], BF)  # phase-3 lhsT tiles
                for cb in range(NC):
                    nc.sync.dma_start(
                        out=woT_sb[:, cb * EQ:(cb + 1) * EQ],
                        in_=woT[cb * 128:(cb + 1) * 128, :],
                    )

                # ---------------- phase 2: attention ----------------
                # per-k-tile pipeline, 5-deep sT PSUM so the exp latency never
                # gates the PE. PSUM: sT 5x1 + oT 2x1 + den 1x1 = 8 banks.
                with (
                    tc.tile_pool(name="ps_sT", bufs=5, space="PSUM") as ps_sT,
                    tc.tile_pool(name="ps_oT", bufs=2, space="PSUM") as ps_oT,
                    tc.tile_pool(name="ps_den", bufs=1, space="PSUM") as ps_den,
                ):
                    for tg in range(NT):
                        nkb = 4 * tg + 4  # causal: k-tiles 0..nkb-1
                        nq = nkb // 4
                        for h in range(HPC):
                            g = h // R  # local kv head
                            qt_h = qt_sb[:, h * T:(h + 1) * T]
                            kt_g = kt_sb[:, g * T:(g + 1) * T]
                            oT_st = work2.tile([128, 512], BF, tag="oTst",
                                               bufs=4)
                            oT_acc = ps_oT.tile([128, 512], F32, tag="oT")
                            den_acc = ps_den.tile([128, 512], F32, tag="den")
                            exps = []
                            for kb in range(nkb):
                                jd = kb - 4 * tg  # diag 128-subtile (if 0..3)
                                js = max(0, jd)  # valid q-cols start at js*128
                                sT = ps_sT.tile([128, 512], F32, tag="sT")
                                nc.tensor.matmul(
                                    sT[:, js * 128:],
                                    kt_g[:, kb * 128:(kb + 1) * 128],
                                    qt_h[:, tg * 512 + js * 128:(tg + 1) * 512],
                                    start=True,
                                    stop=True,
                                )
                                if 0 <= jd < 4:
                                    if jd > 0:
                                        # dead left region <- -1e30 (DVE copy)
                                        nc.vector.tensor_copy(
                                            sT[:, :jd * 128],
                                            mask_sb[:, 128:128 + jd * 128],
                                        )
                                    nc.vector.tensor_tensor(
                                        sT[:, jd * 128:(jd + 1) * 128],
                                        sT[:, jd * 128:(jd + 1) * 128],
                                        mask_sb[:, 0:128],
                                        mybir.AluOpType.add,
                                    )
                                expT = work2.tile([128, 512], BF, tag="expT",
                                                  bufs=8)
                                nc.scalar.activation(
                                    expT[:, js * 128:],
                                    sT[:, js * 128:],
                                    mybir.ActivationFunctionType.Exp,
                                    scale=INV_SQRT_HD,
                                )
                                exps.append(expT)
                                nc.tensor.matmul(
                                    oT_acc[:, js * 128:],
                                    v_sb[:, (g * NKB + kb) * 128:
                                         (g * NKB + kb + 1) * 128],
                                    expT[:, js * 128:],
                                    start=(kb == 0),
                                    stop=(kb == nkb - 1),
                                    skip_group_check=True,
                                )
                                if kb % 4 == 3:
                                    qd = kb // 4
                                    quad = work2.tile([128, 512], BF, tag="dq",
                                                      bufs=3)
                                    if qd == tg:
                                        # diag quad: tiles are narrowed to
                                        # [jd*128:], build the sum tiered
                                        nc.gpsimd.tensor_copy(
                                            quad[:], exps[4 * qd][:]
                                        )
                                        for jj in range(1, 4):
                                            nc.gpsimd.tensor_tensor(
                                                quad[:, jj * 128:],
                                                quad[:, jj * 128:],
                                                exps[4 * qd + jj][:, jj * 128:],
                                                mybir.AluOpType.add,
                                            )
                                    else:
                                        p0 = work2.tile([128, 512], BF,
                                                        tag="dp0", bufs=3)
                                        p1t = work2.tile([128, 512], BF,
                                                         tag="dp1", bufs=3)
                                        nc.gpsimd.tensor_tensor(
                                            p0[:], exps[4 * qd][:],
                                            exps[4 * qd + 1][:],
                                            mybir.AluOpType.add,
                                        )
                                        nc.gpsimd.tensor_tensor(
                                            p1t[:], exps[4 * qd + 2][:],
                                            exps[4 * qd + 3][:],
                                            mybir.AluOpType.add,
                                        )
                                        nc.gpsimd.tensor_tensor(
                                            quad[:], p0[:], p1t[:],
                                            mybir.AluOpType.add,
                                        )
                                    # all-ones stationary: every out partition
                                    # carries the denominator row (broadcast
                                    # comes free with the reduction matmul)
                                    nc.tensor.matmul(
                                        den_acc[:],
                                        ones_sb[:],
                                        quad[:],
                                        start=(qd == 0),
                                        stop=(qd == nq - 1),
                                        skip_group_check=True,
                                    )
                            recip = work2.tile([128, 512], F32, tag="recip",
                                               bufs=2)
                            nc.vector.reciprocal_approx_fast(
                                out=recip[:], in_=den_acc[:]
                            )
                            nc.vector.tensor_tensor(
                                oT_st[:],
                                oT_acc[:],
                                recip[:],
                                mybir.AluOpType.mult,
                            )
                            # ship this (head, tg) chunk and gather peers'
                            nc.sync.dma_start(
                                out=og_in[h][tg][:], in_=oT_st[:]
                            )
                            nc.gpsimd.collective_compute(
                                "AllGather",
                                mybir.AluOpType.bypass,
                                replica_groups=GROUPS,
                                ins=[og_in[h][tg].opt()],
                                outs=[og_out[h][tg].opt()],
                            )

                # ---------------- phase 3: outT slice = WoT.T @ oT_full --------
                # global e-tile eb <-> global head H: rank r = eb//8, local hl = eb%8
                # gathered oT staged through rotating quarter-slabs (read once/tg)
                with (
                    tc.tile_pool(name="og_pool", bufs=6) as og_pool,
                    tc.tile_pool(name="ps_out", bufs=2, space="PSUM") as ps_out,
                ):
                    eb_avail = [rr * HPC + hh
                                for hh in range(HPC) for rr in range(4)]
                    for tg in range(NT):
                        ogq = []
                        for q in range(4):
                            t = og_pool.tile([128, 8, 512], BF, tag="og",
                                             name=f"og_{tg}_{q}")
                            for i in range(8):
                                eb = eb_avail[q * 8 + i]
                                r, hl = eb // HPC, eb % HPC
                                nc.sync.dma_start(
                                    out=t[:, i, :],
                                    in_=og_out[hl][tg][r * 128:(r + 1) * 128, :],
                                )
                            ogq.append(t)
                        for occ in range(2):  # oc chunks of 4
                            accs = []
                            for oi in range(4):
                                acc = ps_out.tile([128, 512], F32, tag=f"out{oi}",
                                                  name=f"p3acc{oi}")
                                accs.append(acc)
                            for ei, eb in enumerate(eb_avail):  # 32 e-tiles
                                rhs_sl = ogq[ei // 8][:, ei % 8, :]
                                for oi in range(4):
                                    oc = occ * 4 + oi
                                    nc.tensor.matmul(
                                        accs[oi][:],
                                        woT_sb[:, eb * EQ + oc * 128:
                                               eb * EQ + (oc + 1) * 128],
                                        rhs_sl,
                                        start=(ei == 0),
                                        stop=(ei == NC - 1),
                                    )
                            for oi in range(4):
                                oc = occ * 4 + oi
                                ev = work2.tile([128, 512], F32, tag="ev")
                                nc.scalar.activation(
                                    ev[:],
                                    accs[oi][:],
                                    mybir.ActivationFunctionType.Copy,
                                )
                                nc.sync.dma_start(
                                    out=out_part[oc * 128:(oc + 1) * 128,
                                                 tg * 512:(tg + 1) * 512],
                                    in_=ev[:],
                                )
    nc.finalize()
    return nc


_NC_CACHE = None


def _get_nc():
    global _NC_CACHE
    if _NC_CACHE is None:
        _NC_CACHE = build()
    return _NC_CACHE


def _make_maskL():
    """[128,512]: cols 0-127 = causal triangle add-mask (allow k-offset r <=
    q-offset c), cols 128-511 = all -1e30 (copy-mask for dead regions)."""
    r = np.arange(128)[:, None]
    c = np.arange(128)[None, :]
    tri = np.where(r <= c, 0.0, -1e30).astype(np.float32)
    flat = np.full((128, 384), -1e30, dtype=np.float32)
    return np.concatenate([tri, flat], axis=1)


def kernel(x, Wq, Wkv, Wo):
    x = np.asarray(x, dtype=np.float32)
    Wq = np.asarray(Wq, dtype=np.float32)
    Wkv = np.asarray(Wkv, dtype=np.float32)
    Wo = np.asarray(Wo, dtype=np.float32)

    # host-side prep (transposes + bf16 casts)
    try:
        import ml_dtypes

        bf16 = ml_dtypes.bfloat16
    except ImportError:  # pragma: no cover
        import jax.numpy as jnp

        bf16 = jnp.bfloat16

    xT_b = [np.ascontiguousarray(x[b].T).astype(bf16) for b in range(B)]

    maskL = _make_maskL()
    ident = np.eye(128, dtype=np.float32).astype(bf16)
    ones = np.ones((128, 128), dtype=np.float32).astype(bf16)

    in_maps = []
    for c in range(NCORES):
        b, j = c // 4, c % 4
        wq_l = Wq[EQ * j:EQ * (j + 1), :]  # [1024, 4096]
        wk_l = Wkv[EKV * j:EKV * (j + 1), :]  # [256, 4096]
        wv_l = Wkv[N_KV * HD + EKV * j:N_KV * HD + EKV * (j + 1), :]
        wall = np.concatenate([wq_l, wk_l, wv_l], axis=0)  # [1536, 4096]
        wallT = np.ascontiguousarray(wall.T).astype(bf16)  # [4096, 1536]
        woT_l = np.ascontiguousarray(Wo[EQ * j:EQ * (j + 1), :].T).astype(bf16)
        in_maps.append(
            {
                "xT": xT_b[b],
                "wallT": wallT,
                "woT": woT_l,
                "maskL": maskL,
                "ident": ident,
                "ones_in": ones,
            }
        )

    nc = _get_nc()
    res = run_bass_kernel_spmd(nc, in_maps, core_ids=list(range(NCORES)))

    out = np.empty((B, T, DIM), dtype=np.float32)
    for b in range(B):
        outT = np.concatenate(
            [res.results[b * 4 + j]["out_part"] for j in range(4)], axis=0
        )  # [4096, 2048]
        out[b] = outT.T
    return out
